# revision 37
# baseline (speedup 1.0000x reference)
"""Multi-head attention (B=4, S=2048, D=1024, H=16) on 8 TRN2 NeuronCores.

Sharding: core c <- batch c//2, heads 8*(c%2) .. 8*(c%2)+8 (Megatron-style:
Wq/Wk/Wv column-parallel, Wo row-parallel). No collectives: the two partial
outputs per batch are summed on the host (plus the bo bias).

Per-core kernel strategy (all matmul operands fp16; host pre-converts to
fully-contiguous DMA layouts):
  - The softmax exp is SPLIT between the scalar engine (ACTIVATE Exp) and
    a custom DVE op (EXP2_BITS_ANT) that constructs the fp16 bit pattern
    of exp(u) arithmetically (magic-number round + quadratic correction,
    ~0.12% rms) and writes it through the int16 value conversion. Wq/bq
    carry the 0.125*1024/ln2 scale so the scores psum is the exp2 argument.
  - PE: concurrency requires disjoint PSUM output partitions, so scores
    (full 128-key outputs) run 2x213ns serial per group while the ctx and
    denominator ones-matmuls run as column-tile pairs (2 MMs / 216ns).
    Groups are emitted as double-bursts (k, k+1) to cut config switches.
  - Projections / output projection stream through a background-work
    generator into the PE slack; explicit add_dep_helper edges cover the
    DVE-write -> matmul-stationary-read hazards Tile misses.
  - Half-boundary: ctx/l psum banks swap tags each half so the new half's
    first matmul reuses the bank freed by the 1-op reciprocal, not the
    3-op normalize chain; exp k=14..15/0..1 stay on ACT to keep the DVE
    queue drained there.
"""
import itertools
import sys

sys.path.insert(0, "/opt/trn_rl_repo")
import numpy as np

import concourse.bass as bass
import concourse.bacc as bacc
import concourse.mybir as mybir
import concourse.tile as tile
from concourse.tile import add_dep_helper
from concourse.bass_utils import run_bass_kernel_spmd

f32 = mybir.dt.float32
f16 = mybir.dt.float16
i16 = mybir.dt.int16
EXP = mybir.ActivationFunctionType.Exp

S = 2048          # sequence length
D = 1024          # model dim
HC = 8            # heads per core
DK = 64           # head dim
JC = HC * DK      # per-core projection width (512)
SCALE = 0.125     # 1/sqrt(DK)
N_CORES = 8

# --- custom DVE exp: fp16-bit-construction exp2 with quadratic correction ---
# Wq/bq are pre-scaled by SCQ so the scores psum holds z = u * 1024/ln2
# (u = raw*0.125, the softmax argument). The op computes the fp16 BIT PATTERN
# of exp(u): p = z + 15360 + corr, written via the int16 value-conversion and
# bitcast back to fp16. corr = A*(f^2 - 512^2), f = z+15872 minus its
# round-to-nearest multiple of 1024 (magic-number trick, C1 = 1.5*2^33).
# Tuned on N(0,1.2) args: mean ~3e-4, rms 0.0019, max 0.003 rel err.
SCQ = 0.125 * 1024.0 / np.log(2.0)        # fold into Wq/bq host-side
SACT = float(np.log(2.0) / 1024.0)        # ACT scale: exp(z*SACT) = exp(u)
EXP2_C0 = 15872.0
EXP2_C1 = float(1.5 * 2.0**33)
EXP2_A = 3.43e-4
EXP2_C3 = 599.79


def _register_exp2_op():
    from concourse import dve_ops
    from concourse.dve_spec import Spec, Src0, C0, C1, C2, C3, lower, _spill_c3_to_src1
    from concourse.dve_uop import DveOpSpec

    if any(op.name == "EXP2_BITS_ANT" for op in dve_ops.OPS):
        return next(op for op in dve_ops.OPS if op.name == "EXP2_BITS_ANT")
    z2 = Src0 + C0
    s = z2 + C1
    r = s - C1
    f = z2 - r
    body = (z2 + (f * f) * C2) - C3

    def ref(in0, in1, s0, s1, imm2):
        z2 = (in0.astype(np.float32) + np.float32(s0)).astype(np.float32)
        s = (z2 + np.float32(s1)).astype(np.float32)
        r = (s - np.float32(s1)).astype(np.float32)
        f = (z2 - r).astype(np.float32)
        w = (z2 + (f * f).astype(np.float32) * np.float32(imm2)).astype(np.float32)
        return (w - in1).astype(np.float32)

    spec = Spec(body=_spill_c3_to_src1(body), reference=ref)
    shas = {
        ver: DveOpSpec(name="EXP2_BITS_ANT", opcode=0, uops=lower(spec, ver=ver),
                       rd1_en=True).sha(ver)
        for ver in ("v3", "v4")
    }
    op = dve_ops.DveOp("EXP2_BITS_ANT", spec, subdim=False, uops_sha=shas)
    dve_ops.OPS.append(op)
    dve_ops._SUB_OPCODE_FOR_NAME[op.name] = (
        dve_ops._CUSTOM_DVE_ROW_BASE + len(dve_ops.OPS) - 1
    )
    assert dve_ops._SUB_OPCODE_FOR_NAME[op.name] < 0x20
    return op


EXP2_OP = _register_exp2_op()

# which k-tiles of each 16-k group run exp on the DVE instead of ACT.
# ~6/16 on the (slower, busier) DVE; k=14..15/0..1 stay on ACT so the DVE
# queue is drained at half boundaries (the ctx/l psum-bank reuse waits on
# the normalize chain queued there). 8/16 measured far worse (DVE exp is
# 1395ns vs ACT 1114ns and DVE carries the normalize/bias work too).
DVE_KS = frozenset({2, 5, 8, 11, 13})


class _NS:
    pass


def build_nc():
    nc = bacc.Bacc(None, target_bir_lowering=False, debug=False)

    io = _NS()
    # all inputs pre-laid-out host-side so every DMA is a contiguous copy
    io.qt = nc.dram_tensor("qt", [2, 128, 8, 1024], f16, kind="ExternalInput")
    io.kt = nc.dram_tensor("kt", [2, 128, 8, 1024], f16, kind="ExternalInput")
    io.vt = nc.dram_tensor("vt", [16, 128, 8, 128], f16, kind="ExternalInput")
    io.wqt = nc.dram_tensor("wqt", [128, 8, JC], f16, kind="ExternalInput")
    io.wkt = nc.dram_tensor("wkt", [128, 8, JC], f16, kind="ExternalInput")
    io.wvt = nc.dram_tensor("wvt", [128, 8, JC], f16, kind="ExternalInput")
    io.wot = nc.dram_tensor("wot", [128, 4, D], f16, kind="ExternalInput")
    io.bq = nc.dram_tensor("bq", [128, 4], f32, kind="ExternalInput")
    io.bk = nc.dram_tensor("bk", [128, 4], f32, kind="ExternalInput")
    io.bvb = nc.dram_tensor("bvb", [128, JC], f32, kind="ExternalInput")
    io.out = nc.dram_tensor("out", [S, D], f16, kind="ExternalOutput")

    with tile.TileContext(nc) as tc:
        with (
            tc.tile_pool(name="big", bufs=1) as big,
            tc.tile_pool(name="work", bufs=3) as work,
            tc.tile_pool(name="xpool", bufs=1) as xp,
            tc.tile_pool(name="xvpool", bufs=4) as xvp,
            tc.tile_pool(name="att", bufs=7) as att,
            tc.tile_pool(name="att2", bufs=2) as att2,
        ):
            sb = _NS()
            sb.qT_sb = big.tile([128, 4, S], f16)           # [p, jt, s]
            sb.kT_sb = big.tile([128, 4, S], f16)
            sb.v_sb = big.tile([128, 16, HC, DK], f16)      # [p, st, h, c]
            sb.ones_sb = big.tile([128, DK], f16)
            sb.wq_sb = big.tile([128, 8, JC], f16)
            sb.wk_sb = big.tile([128, 8, JC], f16)
            sb.wv_sb = big.tile([128, 8, JC], f16)
            sb.bq_sb = big.tile([128, 4], f32)
            sb.bk_sb = big.tile([128, 4], f32)
            sb.bvb_sb = big.tile([128, JC], f32)
            sb.ctxn_sb = big.tile([128, 4, S], f16)         # [p, pair, s]
            sb.wot_sb = big.tile([128, 4, D], f16)
            sb.c3_sb = big.tile([128, 1], f32)              # spilled C3 for EXP2

            nc.vector.memset(sb.ones_sb[:], 1.0)
            nc.vector.memset(sb.c3_sb[:], EXP2_C3)

            bias_insts = {}   # (key, jt, sc) -> bias-add instruction
            mul_insts = {}    # (sqb, pair, half) -> [mul instructions]

            def _dep(reader, writer, why):
                # Tile misses DVE-write -> matmul-stationary-read deps when
                # emission is tightly interleaved; add the edge explicitly.
                f = reader.ins if isinstance(reader, bass.BassInstruction) else reader
                t = writer.ins if isinstance(writer, bass.BassInstruction) else writer
                add_dep_helper(f, t, sync=True, reason=why)

            xq = [None, None]
            xk = [None, None]

            def dma_x(x_dram, tag, sc, ts, split=False):
                t = xp.tile([128, 8, 1024], f16, tag=f"{tag}{sc}")
                if split:
                    # two half transfers so the hf0 projection fill can start
                    # after 1MB instead of 2MB (startup critical path)
                    nc.sync.dma_start(t[:, :, 0:512], x_dram[sc, :, :, 0:512])
                    nc.sync.dma_start(t[:, :, 512:1024],
                                      x_dram[sc, :, :, 512:1024])
                else:
                    nc.sync.dma_start(t[:], x_dram[sc])
                ts[sc] = t

            def qk_fill(pool, tag, key, x_ts, w_sb, o_sb, b_sb, jt, sc,
                        halves=(0, 1)):
                """[128,512] projection half-fills: 1-bank background slots
                that double-buffer."""
                for hf in halves:
                    ps = pool.tile([128, 512], f32, tag=tag,
                                   name=f"pj_{key}_{jt}_{sc}_{hf}")
                    s0 = hf * 512
                    for kt in range(8):
                        w = w_sb[:, kt, jt * 128:(jt + 1) * 128]
                        nc.tensor.matmul(ps[:], w, x_ts[sc][:, kt, s0:s0 + 512],
                                         start=(kt == 0), stop=(kt == 7))
                        if kt % 2:
                            yield
                    bias_insts[(key, jt, sc, hf)] = nc.vector.tensor_scalar_add(
                        o_sb[:, jt, sc * 1024 + s0:sc * 1024 + s0 + 512],
                        ps[:], b_sb[:, jt:jt + 1])
                    yield

            def run(gen):
                for _ in gen:
                    pass

            v_bias = {}

            def v_fill_step(pool, st):
                for _ in v_fill_gen(pool, st):
                    pass

            def v_fill_gen(pool, st, tag="projv"):
                xv = xvp.tile([128, 8, 128], f16, tag="xv")
                nc.sync.dma_start(xv[:], io.vt[st])
                ps = pool.tile([128, JC], f32, tag=tag,
                               name=f"vps_{st}")
                for kt in range(8):
                    nc.tensor.matmul(ps[:], xv[:, kt, :], sb.wv_sb[:, kt, :],
                                     start=(kt == 0), stop=(kt == 7))
                    if kt % 2:
                        yield
                v_bias[st] = nc.vector.tensor_add(
                    sb.v_sb[:, st, :, :],
                    ps[:].rearrange("p (h c) -> p h c", h=HC),
                    sb.bvb_sb[:].rearrange("p (h c) -> p h c", h=HC),
                )
                yield

            # --- upfront: v first; the q/k x/w DMA issues are interleaved
            # into the xv DMA sequence so the Sync queue (which is in-order
            # and paced by the xv slot rotation) doesn't delay them.
            fatv_cm = tc.tile_pool(name="fatv", bufs=2, space="PSUM")
            fatv = fatv_cm.__enter__()
            with (
                tc.tile_pool(name="fat", bufs=2, space="PSUM") as fat,
            ):
                # q/k weights+activations stream FIRST so the scores groups
                # can start ~20us earlier; v st0/st1 follow (their deadline is
                # the ctx matmuls of groups ~0-1, guard-paced), v st2-15 and
                # the k-sc1 chunk go to background.
                nc.sync.dma_start(sb.wq_sb[:], io.wqt[:])
                nc.sync.dma_start(sb.bq_sb[:], io.bq[:])
                nc.sync.dma_start(sb.wk_sb[:], io.wkt[:])
                nc.sync.dma_start(sb.bk_sb[:], io.bk[:])
                # split x transfers, h0 halves first: the first scores burst
                # needs only the hf0 fills of q and k
                xqt = xp.tile([128, 8, 1024], f16, tag="xq0")
                xkt = xp.tile([128, 8, 1024], f16, tag="xk0")
                nc.sync.dma_start(xqt[:, :, 0:512], io.qt[0, :, :, 0:512])
                nc.sync.dma_start(xkt[:, :, 0:512], io.kt[0, :, :, 0:512])
                nc.sync.dma_start(xqt[:, :, 512:1024], io.qt[0, :, :, 512:1024])
                nc.sync.dma_start(xkt[:, :, 512:1024], io.kt[0, :, :, 512:1024])
                xq[0], xk[0] = xqt, xkt
                nc.sync.dma_start(sb.wv_sb[:], io.wvt[:])
                nc.sync.dma_start(sb.bvb_sb[:], io.bvb[:])
                run(qk_fill(fat, "proj", "q", xq, sb.wq_sb, sb.qT_sb, sb.bq_sb,
                            0, 0, halves=(0,)))
                run(qk_fill(fat, "proj", "k", xk, sb.wk_sb, sb.kT_sb, sb.bk_sb,
                            0, 0, halves=(0,)))
                run(qk_fill(fat, "proj", "q", xq, sb.wq_sb, sb.qT_sb, sb.bq_sb,
                            0, 0, halves=(1,)))
                run(qk_fill(fat, "proj", "k", xk, sb.wk_sb, sb.kT_sb, sb.bk_sb,
                            0, 0, halves=(1,)))
                for st in range(2):
                    v_fill_step(fatv, st)
                dma_x(io.kt, "xk", 1, xk)

            # --- stage 2/3 with background stage-1 work -------------------
            with (
                tc.tile_pool(name="ps2st", bufs=2, space="PSUM") as pp_st,
                tc.tile_pool(name="ps2cl", bufs=1, space="PSUM") as pp_cl,
            ):
                pp_pj = fatv
                def stage3_chunk(sq2, tagit):
                    sqb_r, half_r = sq2 // 8, (sq2 % 8) // 4
                    for n in range(2):
                        ps = pp_pj.tile([128, 512], f32, tag=next(tagit),
                                        name=f"o_{sq2}_{n}")
                        for p in range(4):
                            omm = nc.tensor.matmul(
                                ps[:],
                                sb.ctxn_sb[:, p, sq2 * 128:(sq2 + 1) * 128],
                                sb.wot_sb[:, p, n * 512:(n + 1) * 512],
                                start=(p == 0), stop=(p == 3),
                            )
                            if n == 0:
                                for m in mul_insts[(sqb_r, p, half_r)]:
                                    _dep(omm, m, f"out({sq2}) after ctxn")
                            if p % 2:
                                yield
                        ob = work.tile([128, 512], f16, tag="ob")
                        # alternate the psum evacuation between scalar and
                        # vector so neither engine's queue gates the PE
                        if n == 0:
                            nc.scalar.copy(ob[:], ps[:, 0:512])
                        else:
                            nc.vector.tensor_copy(ob[:], ps[:, 0:512])
                        nc.sync.dma_start(
                            io.out[sq2 * 128:(sq2 + 1) * 128,
                                   n * 512:(n + 1) * 512],
                            ob[:],
                        )
                        yield

                def bg_qk():
                    # ordered by deadline: k-jt0-c1 by group 8, v st8-15 by
                    # ~group 2k, pair p (group 32p) needs q-sc0/k-sc0/k-sc1
                    # of jt=p; the q-sc1 fills are only read in s_q block 1.
                    q_args = ("q", xq, sb.wq_sb, sb.qT_sb, sb.bq_sb)
                    k_args = ("k", xk, sb.wk_sb, sb.kT_sb, sb.bk_sb)
                    # v st2-3 first (deadline ~group 2-3), then the k-jt0-c1
                    # fill (group 8), then the rest of the v fills.
                    for st in (2, 3):
                        yield from v_fill_gen(pp_pj, st)
                    yield from qk_fill(pp_pj, "projv", "k", xk, sb.wk_sb,
                                       sb.kT_sb, sb.bk_sb, 0, 1)
                    for st in range(4, 16):
                        yield from v_fill_gen(pp_pj, st)
                    for jt in range(1, 4):
                        for (key, x_ts, w_sb, o_sb, b_sb), sc in (
                            (q_args, 0), (k_args, 0), (k_args, 1),
                        ):
                            yield from qk_fill(pp_pj, "projv", key, x_ts, w_sb,
                                               o_sb, b_sb, jt, sc)
                        if jt == 1:
                            # 6MB that nothing reads before group ~62; keeping
                            # these out of the startup DMA queue lets the
                            # background xv transfers land on time.
                            dma_x(io.qt, "xq", 1, xq)
                            nc.sync.dma_start(
                                sb.wot_sb[:],
                                io.wot[:])
                    for jt in range(0, 4):
                        key, x_ts, w_sb, o_sb, b_sb = q_args
                        yield from qk_fill(pp_pj, "projv", key, x_ts, w_sb,
                                           o_sb, b_sb, jt, 1)

                def bg_s3():
                    # output projection for s_q block 0 (runs during block 1)
                    tagit = itertools.cycle(["projv"])
                    for sq2 in range(8):
                        yield from stage3_chunk(sq2, tagit)

                def bg_s3b():
                    # block-1 rows whose ctxn (half 0) is already complete
                    tagit = itertools.cycle(["projv"])
                    for sq2 in range(8, 12):
                        yield from stage3_chunk(sq2, tagit)

                bgs = [bg_qk()]

                def pump(n=1):
                    done = 0
                    while bgs and done < n:
                        try:
                            next(bgs[0])
                            done += 1
                        except StopIteration:
                            bgs.pop(0)

                state = {}

                def emit_cl(g, pt):
                    sqb, pair, half, k = g
                    ctx, lx = state[(sqb, pair, half)]
                    h0, h1 = 2 * pair, 2 * pair + 1
                    st0, sp0 = (k == 0), (k == 15)
                    if (sqb, pair, half) == (0, 0, 0):
                        # ensure the v projection for this k-tile is emitted,
                        # then guard the stationary read explicitly.
                        while k not in v_bias and bgs:
                            pump(1)
                    cmm = nc.tensor.matmul(ctx[0:64, :], sb.v_sb[:, k, h0, :],
                                     pt[:, 0:512], start=st0, stop=sp0,
                                     skip_group_check=True)
                    if (sqb, pair, half) == (0, 0, 0):
                        _dep(cmm, v_bias[k], f"ctx(k={k}) after v bias")
                    nc.tensor.matmul(ctx[64:128, :], sb.v_sb[:, k, h1, :],
                                     pt[:, 512:1024], start=st0, stop=sp0,
                                     skip_group_check=True)
                    nc.tensor.matmul(lx[0:64, :], sb.ones_sb[:],
                                     pt[:, 0:512], start=st0, stop=sp0,
                                     skip_group_check=True)
                    nc.tensor.matmul(lx[64:128, :], sb.ones_sb[:],
                                     pt[:, 512:1024], start=st0, stop=sp0,
                                     skip_group_check=True)

                def normalize(g):
                    # 3 DVE ops straight off PSUM (no evacuation copies): the
                    # banks free ~0.7us later but the DVE queue sheds 1.4us,
                    # which matters more in the engine-saturated late phase.
                    sqb, pair, half, _ = g
                    ctx, lx = state.pop((sqb, pair, half))
                    sq0 = sqb * 1024 + half * 512
                    r = att2.tile([128, 512], f32, tag="r",
                                  name=f"r_{sqb}_{pair}_{half}")
                    nc.vector.reciprocal_approx_fast(r[:], lx[:])
                    mul_insts[(sqb, pair, half)] = [
                        nc.vector.tensor_mul(
                            sb.ctxn_sb[0:64, pair, sq0:sq0 + 512],
                            ctx[0:64, :], r[0:64, :],
                        ),
                        nc.vector.tensor_mul(
                            sb.ctxn_sb[64:128, pair, sq0:sq0 + 512],
                            ctx[64:128, :], r[64:128, :],
                        ),
                    ]

                groups = [(sqb, pair, half, k)
                          for sqb in range(2) for pair in range(4)
                          for half in range(2) for k in range(16)]
                # ctx/l trail the scores/exp stream by 2 groups so a ctx
                # matmul waiting on the single cl psum slot at a half
                # boundary has two score-groups queued ahead of it (the
                # tensor queue is in-order; a stalled ctx MM would
                # otherwise delay the next scores and gap the ACT).
                # Double-burst: emit scores for (k, k+1) back-to-back, then
                # ctx/l for two pended groups back-to-back. Fewer PE
                # row/col-tile config transitions -> less drain exposure.
                pending = []
                sts = {}
                for bi in range(0, len(groups), 2):
                    burst = groups[bi:bi + 2]
                    for g in burst:
                        sqb, pair, half, k = g
                        if k == 0:
                            if (sqb, pair, half) == (1, 0, 0):
                                bgs.append(bg_s3())
                            # alternate which bank holds ctx vs l: the new
                            # half's FIRST matmul (ctx) then reuses the old
                            # l bank, which frees after 1 DVE op (recip)
                            # instead of 3 (recip+muls) -> shorter boundary
                            # stall.
                            par = (sqb * 8 + pair * 2 + half) % 2
                            tg = ("ctx", "l") if par == 0 else ("l", "ctx")
                            state[(sqb, pair, half)] = (
                                pp_cl.tile([128, 512], f32, tag=tg[0],
                                           name=f"ctx_{sqb}_{pair}_{half}"),
                                pp_cl.tile([128, 512], f32, tag=tg[1],
                                           name=f"l_{sqb}_{pair}_{half}"),
                            )
                        if (sqb, pair, half, k) == (1, 3, 1, 7):
                            bgs.append(bg_s3b())
                        sq0 = sqb * 1024 + half * 512
                        # resolve fill deps BEFORE emitting the pair so bg
                        # pumping never splits the two scores matmuls
                        qkey = kkey = None
                        if k == 0 and not (sqb == 0 and pair == 0):
                            qkey = ("q", pair, sqb, half)
                            while qkey not in bias_insts and bgs:
                                pump(1)
                        if half == 0 and k % 4 == 0 and not (
                                sqb == 0 and pair == 0 and k == 0):
                            kkey = ("k", pair, k // 8, (k // 4) % 2)
                            while kkey not in bias_insts and bgs:
                                pump(1)
                        st = pp_st.tile([128, 1024], f32, tag="st")
                        sts[g] = st
                        smm = nc.tensor.matmul(
                            st[:, 0:512],
                            sb.kT_sb[0:64, pair, k * 128:(k + 1) * 128],
                            sb.qT_sb[0:64, pair, sq0:sq0 + 512],
                            start=True, stop=True,
                        )
                        why = f"scores({sqb},{pair}) after qk bias"
                        if qkey is not None:
                            _dep(smm, bias_insts[qkey], why)
                        if kkey is not None:
                            _dep(smm, bias_insts[kkey], why)
                        nc.tensor.matmul(
                            st[:, 512:1024],
                            sb.kT_sb[64:128, pair, k * 128:(k + 1) * 128],
                            sb.qT_sb[64:128, pair, sq0:sq0 + 512],
                            start=True, stop=True,
                        )
                    for g in burst:
                        sqb, pair, half, k = g
                        st = sts.pop(g)
                        pt = att.tile([128, 1024], f16, tag="pt")
                        if k in DVE_KS:
                            nc.vector._custom_dve(
                                EXP2_OP, out=pt[:].bitcast(i16), in0=st[:],
                                in1=sb.c3_sb[:], s0=EXP2_C0, s1=EXP2_C1,
                                imm2=EXP2_A,
                            )
                        else:
                            nc.scalar.activation(pt[:], st[:], EXP, scale=SACT)
                        pending.append((g, pt))
                    while len(pending) > 4:
                        pg = pending.pop(0)
                        emit_cl(*pg)
                        if pg[0][3] == 15:
                            normalize(pg[0])
                    pump(6 if bi < 16 else 4)
                for pg in pending:
                    emit_cl(*pg)
                    if pg[0][3] == 15:
                        normalize(pg[0])

                # drain any remaining background work
                while bgs:
                    try:
                        next(bgs[0])
                    except StopIteration:
                        bgs.pop(0)

                # final output rows (need the very last ctxn half): give every
                # chunk its own psum slot (scores banks are free now) so all
                # 32 matmuls stream back-to-back, with one [128,1024] DMA per
                # row block instead of two.
                st_f0 = pp_st.tile([128, 1024], f32, tag="st")
                st_f1 = pp_st.tile([128, 1024], f32, tag="st")
                slots = [
                    st_f0[:, 0:512], st_f0[:, 512:1024],
                    st_f1[:, 0:512], st_f1[:, 512:1024],
                    pp_pj.tile([128, 512], f32, tag="projv", name="o_f4")[:],
                    pp_pj.tile([128, 512], f32, tag="projv", name="o_f5")[:],
                    pp_cl.tile([128, 512], f32, tag="ctx", name="o_f6")[:],
                    pp_cl.tile([128, 512], f32, tag="l", name="o_f7")[:],
                ]
                # pass 1: p=0..2 accumulations (independent of the very last
                # normalize) so the PE streams while the DVE finishes it;
                # pass 2: the p=3 finishers + evacuation.
                chunks = [(s, n) for s in range(12, 16) for n in range(2)]
                for ci, (sq2, n) in enumerate(chunks):
                    ps = slots[ci]
                    for p in range(3):
                        omm = nc.tensor.matmul(
                            ps,
                            sb.ctxn_sb[:, p, sq2 * 128:(sq2 + 1) * 128],
                            sb.wot_sb[:, p, n * 512:(n + 1) * 512],
                            start=(p == 0), stop=False,
                        )
                        if n == 0:
                            for m in mul_insts[(sq2 // 8, p, (sq2 % 8) // 4)]:
                                _dep(omm, m, f"out({sq2}) after ctxn")
                for ci, (sq2, n) in enumerate(chunks):
                    ps = slots[ci]
                    omm = nc.tensor.matmul(
                        ps,
                        sb.ctxn_sb[:, 3, sq2 * 128:(sq2 + 1) * 128],
                        sb.wot_sb[:, 3, n * 512:(n + 1) * 512],
                        start=False, stop=True,
                    )
                    if n == 0:
                        for m in mul_insts[(sq2 // 8, 3, (sq2 % 8) // 4)]:
                            _dep(omm, m, f"out({sq2}) after ctxn")
                    # both scalar and vector are mostly idle in the tail;
                    # alternate engines and DMA each half as soon as it copies
                    obf = work.tile([128, 512], f16, tag="obf",
                                    name=f"obf_{sq2}_{n}")
                    if n == 0:
                        nc.scalar.copy(obf[:], ps)
                    else:
                        nc.vector.tensor_copy(obf[:], ps)
                    nc.sync.dma_start(
                        io.out[sq2 * 128:(sq2 + 1) * 128,
                               n * 512:(n + 1) * 512],
                        obf[:],
                    )

            fatv_cm.__exit__(None, None, None)

    nc.compile()
    return nc


_NC = None


def _get_nc():
    global _NC
    if _NC is None:
        _NC = build_nc()
    return _NC


def make_in_maps(Q, K, V, Wq, bq, Wk, bk, Wv, bv, Wo, bo):
    ash = lambda x: np.ascontiguousarray(np.asarray(x, dtype=np.float32).astype(np.float16))
    asf = lambda x: np.ascontiguousarray(np.asarray(x, dtype=np.float32))
    in_maps = []
    for c in range(N_CORES):
        b = c // 2
        j0 = JC * (c % 2)
        jsl = slice(j0, j0 + JC)
        xl = lambda x: x.reshape(8, 128, 2, 1024).transpose(2, 1, 0, 3)
        wl = lambda w: w.reshape(8, 128, 512).transpose(1, 0, 2)
        in_maps.append({
            "qt": ash(xl(np.asarray(Q)[b].T)),
            "kt": ash(xl(np.asarray(K)[b].T)),
            "vt": ash(np.asarray(V)[b].T.reshape(8, 128, 16, 128)
                      .transpose(2, 1, 0, 3)),
            "wqt": ash(wl(np.asarray(Wq)[jsl].T * np.float32(SCQ))),
            "wkt": ash(wl(np.asarray(Wk)[jsl].T)),
            "wvt": ash(wl(np.asarray(Wv)[jsl].T)),
            "wot": ash(np.asarray(Wo)[:, jsl].T.reshape(4, 128, 1024)
                       .transpose(1, 0, 2)),
            "bq": asf(np.asarray(bq)[jsl].reshape(4, 128).T * np.float32(SCQ)),
            "bk": asf(np.asarray(bk)[jsl].reshape(4, 128).T),
            "bvb": asf(np.broadcast_to(np.asarray(bv)[jsl], (128, JC))),
        })
    return in_maps


def kernel(Q, K, V, Wq, bq, Wk, bk, Wv, bv, Wo, bo, _trace=False, _trace_kwargs=None):
    nc = _get_nc()
    in_maps = make_in_maps(Q, K, V, Wq, bq, Wk, bk, Wv, bv, Wo, bo)
    res = run_bass_kernel_spmd(
        nc, in_maps, core_ids=list(range(N_CORES)),
        trace=_trace, **(_trace_kwargs or {}),
    )
    parts = [res.results[c]["out"].astype(np.float32) for c in range(N_CORES)]
    bo_np = np.asarray(bo, dtype=np.float32)
    O = np.stack([parts[2 * b] + parts[2 * b + 1] + bo_np for b in range(4)])
    kernel.last_results = res
    return O.astype(np.float32)



# revision 40
# speedup vs baseline: 1.0021x; 1.0021x over previous
"""Multi-head attention (B=4, S=2048, D=1024, H=16) on 8 TRN2 NeuronCores.

Sharding: core c <- batch c//2, heads 8*(c%2) .. 8*(c%2)+8 (Megatron-style:
Wq/Wk/Wv column-parallel, Wo row-parallel). No collectives: the two partial
outputs per batch are summed on the host (plus the bo bias).

Per-core kernel strategy (all matmul operands fp16; host pre-converts to
fully-contiguous DMA layouts):
  - The softmax exp is SPLIT between the scalar engine (ACTIVATE Exp) and
    a custom DVE op (EXP2_BITS_ANT) that constructs the fp16 bit pattern
    of exp(u) arithmetically (magic-number round + quadratic correction,
    ~0.12% rms) and writes it through the int16 value conversion. Wq/bq
    carry the 0.125*1024/ln2 scale so the scores psum is the exp2 argument.
  - PE: concurrency requires disjoint PSUM output partitions, so scores
    (full 128-key outputs) run 2x213ns serial per group while the ctx and
    denominator ones-matmuls run as column-tile pairs (2 MMs / 216ns).
    Groups are emitted as double-bursts (k, k+1) to cut config switches.
  - Projections / output projection stream through a background-work
    generator into the PE slack; explicit add_dep_helper edges cover the
    DVE-write -> matmul-stationary-read hazards Tile misses.
  - Half-boundary: ctx/l psum banks swap tags each half so the new half's
    first matmul reuses the bank freed by the 1-op reciprocal, not the
    3-op normalize chain; exp k=14..15/0..1 stay on ACT to keep the DVE
    queue drained there.
"""
import itertools
import sys

sys.path.insert(0, "/opt/trn_rl_repo")
import numpy as np

import concourse.bass as bass
import concourse.bacc as bacc
import concourse.mybir as mybir
import concourse.tile as tile
from concourse.tile import add_dep_helper
from concourse.bass_utils import run_bass_kernel_spmd

f32 = mybir.dt.float32
f16 = mybir.dt.float16
i16 = mybir.dt.int16
EXP = mybir.ActivationFunctionType.Exp

S = 2048          # sequence length
D = 1024          # model dim
HC = 8            # heads per core
DK = 64           # head dim
JC = HC * DK      # per-core projection width (512)
SCALE = 0.125     # 1/sqrt(DK)
N_CORES = 8

# --- custom DVE exp: fp16-bit-construction exp2 with quadratic correction ---
# Wq/bq are pre-scaled by SCQ so the scores psum holds z = u * 1024/ln2
# (u = raw*0.125, the softmax argument). The op computes the fp16 BIT PATTERN
# of exp(u): p = z + 15360 + corr, written via the int16 value-conversion and
# bitcast back to fp16. corr = A*(f^2 - 512^2), f = z+15872 minus its
# round-to-nearest multiple of 1024 (magic-number trick, C1 = 1.5*2^33).
# Tuned on N(0,1.2) args: mean ~3e-4, rms 0.0019, max 0.003 rel err.
SCQ = 0.125 * 1024.0 / np.log(2.0)        # fold into Wq/bq host-side
SACT = float(np.log(2.0) / 1024.0)        # ACT scale: exp(z*SACT) = exp(u)
EXP2_C0 = 15872.0
EXP2_C1 = float(1.5 * 2.0**33)
EXP2_A = 3.43e-4
EXP2_C3 = 599.79


def _register_exp2_op():
    from concourse import dve_ops
    from concourse.dve_spec import Spec, Src0, C0, C1, C2, C3, lower, _spill_c3_to_src1
    from concourse.dve_uop import DveOpSpec

    if any(op.name == "EXP2_BITS_ANT" for op in dve_ops.OPS):
        return next(op for op in dve_ops.OPS if op.name == "EXP2_BITS_ANT")
    z2 = Src0 + C0
    s = z2 + C1
    r = s - C1
    f = z2 - r
    body = (z2 + (f * f) * C2) - C3

    def ref(in0, in1, s0, s1, imm2):
        z2 = (in0.astype(np.float32) + np.float32(s0)).astype(np.float32)
        s = (z2 + np.float32(s1)).astype(np.float32)
        r = (s - np.float32(s1)).astype(np.float32)
        f = (z2 - r).astype(np.float32)
        w = (z2 + (f * f).astype(np.float32) * np.float32(imm2)).astype(np.float32)
        return (w - in1).astype(np.float32)

    spec = Spec(body=_spill_c3_to_src1(body), reference=ref)
    shas = {
        ver: DveOpSpec(name="EXP2_BITS_ANT", opcode=0, uops=lower(spec, ver=ver),
                       rd1_en=True).sha(ver)
        for ver in ("v3", "v4")
    }
    op = dve_ops.DveOp("EXP2_BITS_ANT", spec, subdim=False, uops_sha=shas)
    dve_ops.OPS.append(op)
    dve_ops._SUB_OPCODE_FOR_NAME[op.name] = (
        dve_ops._CUSTOM_DVE_ROW_BASE + len(dve_ops.OPS) - 1
    )
    assert dve_ops._SUB_OPCODE_FOR_NAME[op.name] < 0x20
    return op


EXP2_OP = _register_exp2_op()

# which k-tiles of each 16-k group run exp on the DVE instead of ACT.
# ~6/16 on the (slower, busier) DVE; k=14..15/0..1 stay on ACT so the DVE
# queue is drained at half boundaries (the ctx/l psum-bank reuse waits on
# the normalize chain queued there). 8/16 measured far worse (DVE exp is
# 1395ns vs ACT 1114ns and DVE carries the normalize/bias work too).
# all burst-FIRST tiles (k even): the slower DVE exp then has a full extra
# scores-pair of slack before its st bank is reused by the next-next burst
DVE_KS = frozenset({2, 4, 8, 10, 12})


class _NS:
    pass


def build_nc():
    nc = bacc.Bacc(None, target_bir_lowering=False, debug=False)

    io = _NS()
    # all inputs pre-laid-out host-side so every DMA is a contiguous copy
    io.qt = nc.dram_tensor("qt", [2, 128, 8, 1024], f16, kind="ExternalInput")
    io.kt = nc.dram_tensor("kt", [2, 128, 8, 1024], f16, kind="ExternalInput")
    io.vt = nc.dram_tensor("vt", [16, 128, 8, 128], f16, kind="ExternalInput")
    io.wqt = nc.dram_tensor("wqt", [128, 8, JC], f16, kind="ExternalInput")
    io.wkt = nc.dram_tensor("wkt", [128, 8, JC], f16, kind="ExternalInput")
    io.wvt = nc.dram_tensor("wvt", [128, 8, JC], f16, kind="ExternalInput")
    io.wot = nc.dram_tensor("wot", [128, 4, D], f16, kind="ExternalInput")
    io.bq = nc.dram_tensor("bq", [128, 4], f32, kind="ExternalInput")
    io.bk = nc.dram_tensor("bk", [128, 4], f32, kind="ExternalInput")
    io.bvb = nc.dram_tensor("bvb", [128, JC], f32, kind="ExternalInput")
    io.out = nc.dram_tensor("out", [S, D], f16, kind="ExternalOutput")

    with tile.TileContext(nc) as tc:
        with (
            tc.tile_pool(name="big", bufs=1) as big,
            tc.tile_pool(name="work", bufs=3) as work,
            tc.tile_pool(name="xpool", bufs=1) as xp,
            tc.tile_pool(name="xvpool", bufs=4) as xvp,
            tc.tile_pool(name="att", bufs=7) as att,
            tc.tile_pool(name="att2", bufs=2) as att2,
        ):
            sb = _NS()
            sb.qT_sb = big.tile([128, 4, S], f16)           # [p, jt, s]
            sb.kT_sb = big.tile([128, 4, S], f16)
            sb.v_sb = big.tile([128, 16, HC, DK], f16)      # [p, st, h, c]
            sb.ones_sb = big.tile([128, DK], f16)
            sb.wq_sb = big.tile([128, 8, JC], f16)
            sb.wk_sb = big.tile([128, 8, JC], f16)
            sb.wv_sb = big.tile([128, 8, JC], f16)
            sb.bq_sb = big.tile([128, 4], f32)
            sb.bk_sb = big.tile([128, 4], f32)
            sb.bvb_sb = big.tile([128, JC], f32)
            sb.ctxn_sb = big.tile([128, 4, S], f16)         # [p, pair, s]
            sb.wot_sb = big.tile([128, 4, D], f16)
            sb.c3_sb = big.tile([128, 1], f32)              # spilled C3 for EXP2

            nc.vector.memset(sb.ones_sb[:], 1.0)
            nc.vector.memset(sb.c3_sb[:], EXP2_C3)

            bias_insts = {}   # (key, jt, sc) -> bias-add instruction
            mul_insts = {}    # (sqb, pair, half) -> [mul instructions]

            def _dep(reader, writer, why):
                # Tile misses DVE-write -> matmul-stationary-read deps when
                # emission is tightly interleaved; add the edge explicitly.
                f = reader.ins if isinstance(reader, bass.BassInstruction) else reader
                t = writer.ins if isinstance(writer, bass.BassInstruction) else writer
                add_dep_helper(f, t, sync=True, reason=why)

            xq = [None, None]
            xk = [None, None]

            def dma_x(x_dram, tag, sc, ts, split=False):
                t = xp.tile([128, 8, 1024], f16, tag=f"{tag}{sc}")
                if split:
                    # two half transfers so the hf0 projection fill can start
                    # after 1MB instead of 2MB (startup critical path)
                    nc.sync.dma_start(t[:, :, 0:512], x_dram[sc, :, :, 0:512])
                    nc.sync.dma_start(t[:, :, 512:1024],
                                      x_dram[sc, :, :, 512:1024])
                else:
                    nc.sync.dma_start(t[:], x_dram[sc])
                ts[sc] = t

            def qk_fill(pool, tag, key, x_ts, w_sb, o_sb, b_sb, jt, sc,
                        halves=(0, 1)):
                """[128,512] projection half-fills: 1-bank background slots
                that double-buffer."""
                for hf in halves:
                    ps = pool.tile([128, 512], f32, tag=tag,
                                   name=f"pj_{key}_{jt}_{sc}_{hf}")
                    s0 = hf * 512
                    for kt in range(8):
                        w = w_sb[:, kt, jt * 128:(jt + 1) * 128]
                        nc.tensor.matmul(ps[:], w, x_ts[sc][:, kt, s0:s0 + 512],
                                         start=(kt == 0), stop=(kt == 7))
                        if kt % 2:
                            yield
                    bias_insts[(key, jt, sc, hf)] = nc.vector.tensor_scalar_add(
                        o_sb[:, jt, sc * 1024 + s0:sc * 1024 + s0 + 512],
                        ps[:], b_sb[:, jt:jt + 1])
                    yield

            def run(gen):
                for _ in gen:
                    pass

            v_bias = {}

            def v_fill_step(pool, st):
                for _ in v_fill_gen(pool, st):
                    pass

            def v_fill_gen(pool, st, tag="projv"):
                xv = xvp.tile([128, 8, 128], f16, tag="xv")
                nc.sync.dma_start(xv[:], io.vt[st])
                ps = pool.tile([128, JC], f32, tag=tag,
                               name=f"vps_{st}")
                for kt in range(8):
                    nc.tensor.matmul(ps[:], xv[:, kt, :], sb.wv_sb[:, kt, :],
                                     start=(kt == 0), stop=(kt == 7))
                    if kt % 2:
                        yield
                v_bias[st] = nc.vector.tensor_add(
                    sb.v_sb[:, st, :, :],
                    ps[:].rearrange("p (h c) -> p h c", h=HC),
                    sb.bvb_sb[:].rearrange("p (h c) -> p h c", h=HC),
                )
                yield

            # --- upfront: v first; the q/k x/w DMA issues are interleaved
            # into the xv DMA sequence so the Sync queue (which is in-order
            # and paced by the xv slot rotation) doesn't delay them.
            fatv_cm = tc.tile_pool(name="fatv", bufs=2, space="PSUM")
            fatv = fatv_cm.__enter__()
            with (
                tc.tile_pool(name="fat", bufs=2, space="PSUM") as fat,
            ):
                # q/k weights+activations stream FIRST so the scores groups
                # can start ~20us earlier; v st0/st1 follow (their deadline is
                # the ctx matmuls of groups ~0-1, guard-paced), v st2-15 and
                # the k-sc1 chunk go to background.
                nc.sync.dma_start(sb.wq_sb[:], io.wqt[:])
                nc.sync.dma_start(sb.bq_sb[:], io.bq[:])
                nc.sync.dma_start(sb.wk_sb[:], io.wkt[:])
                nc.sync.dma_start(sb.bk_sb[:], io.bk[:])
                # split x transfers, h0 halves first: the first scores burst
                # needs only the hf0 fills of q and k
                xqt = xp.tile([128, 8, 1024], f16, tag="xq0")
                xkt = xp.tile([128, 8, 1024], f16, tag="xk0")
                nc.sync.dma_start(xqt[:, :, 0:512], io.qt[0, :, :, 0:512])
                nc.sync.dma_start(xkt[:, :, 0:512], io.kt[0, :, :, 0:512])
                nc.sync.dma_start(xqt[:, :, 512:1024], io.qt[0, :, :, 512:1024])
                nc.sync.dma_start(xkt[:, :, 512:1024], io.kt[0, :, :, 512:1024])
                xq[0], xk[0] = xqt, xkt
                nc.sync.dma_start(sb.wv_sb[:], io.wvt[:])
                nc.sync.dma_start(sb.bvb_sb[:], io.bvb[:])
                run(qk_fill(fat, "proj", "q", xq, sb.wq_sb, sb.qT_sb, sb.bq_sb,
                            0, 0, halves=(0,)))
                run(qk_fill(fat, "proj", "k", xk, sb.wk_sb, sb.kT_sb, sb.bk_sb,
                            0, 0, halves=(0,)))
                run(qk_fill(fat, "proj", "q", xq, sb.wq_sb, sb.qT_sb, sb.bq_sb,
                            0, 0, halves=(1,)))
                run(qk_fill(fat, "proj", "k", xk, sb.wk_sb, sb.kT_sb, sb.bk_sb,
                            0, 0, halves=(1,)))
                for st in range(2):
                    v_fill_step(fatv, st)
                dma_x(io.kt, "xk", 1, xk)

            # --- stage 2/3 with background stage-1 work -------------------
            with (
                tc.tile_pool(name="ps2st", bufs=2, space="PSUM") as pp_st,
                tc.tile_pool(name="ps2cl", bufs=1, space="PSUM") as pp_cl,
            ):
                pp_pj = fatv
                def stage3_chunk(sq2, tagit):
                    sqb_r, half_r = sq2 // 8, (sq2 % 8) // 4
                    for n in range(2):
                        ps = pp_pj.tile([128, 512], f32, tag=next(tagit),
                                        name=f"o_{sq2}_{n}")
                        for p in range(4):
                            omm = nc.tensor.matmul(
                                ps[:],
                                sb.ctxn_sb[:, p, sq2 * 128:(sq2 + 1) * 128],
                                sb.wot_sb[:, p, n * 512:(n + 1) * 512],
                                start=(p == 0), stop=(p == 3),
                            )
                            if n == 0:
                                for m in mul_insts[(sqb_r, p, half_r)]:
                                    _dep(omm, m, f"out({sq2}) after ctxn")
                            if p % 2:
                                yield
                        ob = work.tile([128, 512], f16, tag="ob")
                        # alternate the psum evacuation between scalar and
                        # vector so neither engine's queue gates the PE
                        if n == 0:
                            nc.scalar.copy(ob[:], ps[:, 0:512])
                        else:
                            nc.vector.tensor_copy(ob[:], ps[:, 0:512])
                        nc.sync.dma_start(
                            io.out[sq2 * 128:(sq2 + 1) * 128,
                                   n * 512:(n + 1) * 512],
                            ob[:],
                        )
                        yield

                def bg_qk():
                    # ordered by deadline: k-jt0-c1 by group 8, v st8-15 by
                    # ~group 2k, pair p (group 32p) needs q-sc0/k-sc0/k-sc1
                    # of jt=p; the q-sc1 fills are only read in s_q block 1.
                    q_args = ("q", xq, sb.wq_sb, sb.qT_sb, sb.bq_sb)
                    k_args = ("k", xk, sb.wk_sb, sb.kT_sb, sb.bk_sb)
                    # v st2-3 first (deadline ~group 2-3), then the k-jt0-c1
                    # fill (group 8), then the rest of the v fills.
                    for st in (2, 3):
                        yield from v_fill_gen(pp_pj, st)
                    yield from qk_fill(pp_pj, "projv", "k", xk, sb.wk_sb,
                                       sb.kT_sb, sb.bk_sb, 0, 1)
                    for st in range(4, 16):
                        yield from v_fill_gen(pp_pj, st)
                    for jt in range(1, 4):
                        for (key, x_ts, w_sb, o_sb, b_sb), sc in (
                            (q_args, 0), (k_args, 0), (k_args, 1),
                        ):
                            yield from qk_fill(pp_pj, "projv", key, x_ts, w_sb,
                                               o_sb, b_sb, jt, sc)
                        if jt == 1:
                            # 6MB that nothing reads before group ~62; keeping
                            # these out of the startup DMA queue lets the
                            # background xv transfers land on time.
                            dma_x(io.qt, "xq", 1, xq)
                            nc.sync.dma_start(
                                sb.wot_sb[:],
                                io.wot[:])
                    for jt in range(0, 4):
                        key, x_ts, w_sb, o_sb, b_sb = q_args
                        yield from qk_fill(pp_pj, "projv", key, x_ts, w_sb,
                                           o_sb, b_sb, jt, 1)

                def bg_s3():
                    # output projection for s_q block 0 (runs during block 1)
                    tagit = itertools.cycle(["projv"])
                    for sq2 in range(8):
                        yield from stage3_chunk(sq2, tagit)

                def bg_s3b():
                    # block-1 rows whose ctxn (half 0) is already complete
                    tagit = itertools.cycle(["projv"])
                    for sq2 in range(8, 12):
                        yield from stage3_chunk(sq2, tagit)

                bgs = [bg_qk()]

                def pump(n=1):
                    done = 0
                    while bgs and done < n:
                        try:
                            next(bgs[0])
                            done += 1
                        except StopIteration:
                            bgs.pop(0)

                state = {}

                def emit_cl(g, pt):
                    sqb, pair, half, k = g
                    ctx, lx = state[(sqb, pair, half)]
                    h0, h1 = 2 * pair, 2 * pair + 1
                    st0, sp0 = (k == 0), (k == 15)
                    if (sqb, pair, half) == (0, 0, 0):
                        # ensure the v projection for this k-tile is emitted,
                        # then guard the stationary read explicitly.
                        while k not in v_bias and bgs:
                            pump(1)
                    cmm = nc.tensor.matmul(ctx[0:64, :], sb.v_sb[:, k, h0, :],
                                     pt[:, 0:512], start=st0, stop=sp0,
                                     skip_group_check=True)
                    if (sqb, pair, half) == (0, 0, 0):
                        _dep(cmm, v_bias[k], f"ctx(k={k}) after v bias")
                    nc.tensor.matmul(ctx[64:128, :], sb.v_sb[:, k, h1, :],
                                     pt[:, 512:1024], start=st0, stop=sp0,
                                     skip_group_check=True)
                    nc.tensor.matmul(lx[0:64, :], sb.ones_sb[:],
                                     pt[:, 0:512], start=st0, stop=sp0,
                                     skip_group_check=True)
                    nc.tensor.matmul(lx[64:128, :], sb.ones_sb[:],
                                     pt[:, 512:1024], start=st0, stop=sp0,
                                     skip_group_check=True)

                def normalize(g):
                    # 3 DVE ops straight off PSUM (no evacuation copies): the
                    # banks free ~0.7us later but the DVE queue sheds 1.4us,
                    # which matters more in the engine-saturated late phase.
                    sqb, pair, half, _ = g
                    ctx, lx = state.pop((sqb, pair, half))
                    sq0 = sqb * 1024 + half * 512
                    r = att2.tile([128, 512], f32, tag="r",
                                  name=f"r_{sqb}_{pair}_{half}")
                    nc.vector.reciprocal_approx_fast(r[:], lx[:])
                    mul_insts[(sqb, pair, half)] = [
                        nc.vector.tensor_mul(
                            sb.ctxn_sb[0:64, pair, sq0:sq0 + 512],
                            ctx[0:64, :], r[0:64, :],
                        ),
                        nc.vector.tensor_mul(
                            sb.ctxn_sb[64:128, pair, sq0:sq0 + 512],
                            ctx[64:128, :], r[64:128, :],
                        ),
                    ]

                groups = [(sqb, pair, half, k)
                          for sqb in range(2) for pair in range(4)
                          for half in range(2) for k in range(16)]
                # ctx/l trail the scores/exp stream by 2 groups so a ctx
                # matmul waiting on the single cl psum slot at a half
                # boundary has two score-groups queued ahead of it (the
                # tensor queue is in-order; a stalled ctx MM would
                # otherwise delay the next scores and gap the ACT).
                # Double-burst: emit scores for (k, k+1) back-to-back, then
                # ctx/l for two pended groups back-to-back. Fewer PE
                # row/col-tile config transitions -> less drain exposure.
                pending = []
                sts = {}
                for bi in range(0, len(groups), 2):
                    burst = groups[bi:bi + 2]
                    for g in burst:
                        sqb, pair, half, k = g
                        if k == 0:
                            if (sqb, pair, half) == (1, 0, 0):
                                bgs.append(bg_s3())
                            # alternate which bank holds ctx vs l: the new
                            # half's FIRST matmul (ctx) then reuses the old
                            # l bank, which frees after 1 DVE op (recip)
                            # instead of 3 (recip+muls) -> shorter boundary
                            # stall.
                            par = (sqb * 8 + pair * 2 + half) % 2
                            tg = ("ctx", "l") if par == 0 else ("l", "ctx")
                            state[(sqb, pair, half)] = (
                                pp_cl.tile([128, 512], f32, tag=tg[0],
                                           name=f"ctx_{sqb}_{pair}_{half}"),
                                pp_cl.tile([128, 512], f32, tag=tg[1],
                                           name=f"l_{sqb}_{pair}_{half}"),
                            )
                        if (sqb, pair, half, k) == (1, 3, 1, 7):
                            bgs.append(bg_s3b())
                        sq0 = sqb * 1024 + half * 512
                        # resolve fill deps BEFORE emitting the pair so bg
                        # pumping never splits the two scores matmuls
                        qkey = kkey = None
                        if k == 0 and not (sqb == 0 and pair == 0):
                            qkey = ("q", pair, sqb, half)
                            while qkey not in bias_insts and bgs:
                                pump(1)
                        if half == 0 and k % 4 == 0 and not (
                                sqb == 0 and pair == 0 and k == 0):
                            kkey = ("k", pair, k // 8, (k // 4) % 2)
                            while kkey not in bias_insts and bgs:
                                pump(1)
                        st = pp_st.tile([128, 1024], f32, tag="st")
                        sts[g] = st
                        smm = nc.tensor.matmul(
                            st[:, 0:512],
                            sb.kT_sb[0:64, pair, k * 128:(k + 1) * 128],
                            sb.qT_sb[0:64, pair, sq0:sq0 + 512],
                            start=True, stop=True,
                        )
                        why = f"scores({sqb},{pair}) after qk bias"
                        if qkey is not None:
                            _dep(smm, bias_insts[qkey], why)
                        if kkey is not None:
                            _dep(smm, bias_insts[kkey], why)
                        nc.tensor.matmul(
                            st[:, 512:1024],
                            sb.kT_sb[64:128, pair, k * 128:(k + 1) * 128],
                            sb.qT_sb[64:128, pair, sq0:sq0 + 512],
                            start=True, stop=True,
                        )
                    for g in burst:
                        sqb, pair, half, k = g
                        st = sts.pop(g)
                        pt = att.tile([128, 1024], f16, tag="pt")
                        if k in DVE_KS:
                            nc.vector._custom_dve(
                                EXP2_OP, out=pt[:].bitcast(i16), in0=st[:],
                                in1=sb.c3_sb[:], s0=EXP2_C0, s1=EXP2_C1,
                                imm2=EXP2_A,
                            )
                        else:
                            nc.scalar.activation(pt[:], st[:], EXP, scale=SACT)
                        pending.append((g, pt))
                    while len(pending) > 4:
                        pg = pending.pop(0)
                        emit_cl(*pg)
                        if pg[0][3] == 15:
                            normalize(pg[0])
                    pump(6 if bi < 16 else 4)
                for pg in pending:
                    emit_cl(*pg)
                    if pg[0][3] == 15:
                        normalize(pg[0])

                # drain any remaining background work
                while bgs:
                    try:
                        next(bgs[0])
                    except StopIteration:
                        bgs.pop(0)

                # final output rows (need the very last ctxn half): give every
                # chunk its own psum slot (scores banks are free now) so all
                # 32 matmuls stream back-to-back, with one [128,1024] DMA per
                # row block instead of two.
                st_f0 = pp_st.tile([128, 1024], f32, tag="st")
                st_f1 = pp_st.tile([128, 1024], f32, tag="st")
                slots = [
                    st_f0[:, 0:512], st_f0[:, 512:1024],
                    st_f1[:, 0:512], st_f1[:, 512:1024],
                    pp_pj.tile([128, 512], f32, tag="projv", name="o_f4")[:],
                    pp_pj.tile([128, 512], f32, tag="projv", name="o_f5")[:],
                    pp_cl.tile([128, 512], f32, tag="ctx", name="o_f6")[:],
                    pp_cl.tile([128, 512], f32, tag="l", name="o_f7")[:],
                ]
                # pass 1: p=0..2 accumulations (independent of the very last
                # normalize) so the PE streams while the DVE finishes it;
                # pass 2: the p=3 finishers + evacuation.
                chunks = [(s, n) for s in range(12, 16) for n in range(2)]
                for ci, (sq2, n) in enumerate(chunks):
                    ps = slots[ci]
                    for p in range(3):
                        omm = nc.tensor.matmul(
                            ps,
                            sb.ctxn_sb[:, p, sq2 * 128:(sq2 + 1) * 128],
                            sb.wot_sb[:, p, n * 512:(n + 1) * 512],
                            start=(p == 0), stop=False,
                        )
                        if n == 0:
                            for m in mul_insts[(sq2 // 8, p, (sq2 % 8) // 4)]:
                                _dep(omm, m, f"out({sq2}) after ctxn")
                for ci, (sq2, n) in enumerate(chunks):
                    ps = slots[ci]
                    omm = nc.tensor.matmul(
                        ps,
                        sb.ctxn_sb[:, 3, sq2 * 128:(sq2 + 1) * 128],
                        sb.wot_sb[:, 3, n * 512:(n + 1) * 512],
                        start=False, stop=True,
                    )
                    if n == 0:
                        for m in mul_insts[(sq2 // 8, 3, (sq2 % 8) // 4)]:
                            _dep(omm, m, f"out({sq2}) after ctxn")
                    # both scalar and vector are mostly idle in the tail;
                    # alternate engines and DMA each half as soon as it copies
                    obf = work.tile([128, 512], f16, tag="obf",
                                    name=f"obf_{sq2}_{n}")
                    if n == 0:
                        nc.scalar.copy(obf[:], ps)
                    else:
                        nc.vector.tensor_copy(obf[:], ps)
                    nc.sync.dma_start(
                        io.out[sq2 * 128:(sq2 + 1) * 128,
                               n * 512:(n + 1) * 512],
                        obf[:],
                    )

            fatv_cm.__exit__(None, None, None)

    nc.compile()
    return nc


_NC = None


def _get_nc():
    global _NC
    if _NC is None:
        _NC = build_nc()
    return _NC


def make_in_maps(Q, K, V, Wq, bq, Wk, bk, Wv, bv, Wo, bo):
    ash = lambda x: np.ascontiguousarray(np.asarray(x, dtype=np.float32).astype(np.float16))
    asf = lambda x: np.ascontiguousarray(np.asarray(x, dtype=np.float32))
    in_maps = []
    for c in range(N_CORES):
        b = c // 2
        j0 = JC * (c % 2)
        jsl = slice(j0, j0 + JC)
        xl = lambda x: x.reshape(8, 128, 2, 1024).transpose(2, 1, 0, 3)
        wl = lambda w: w.reshape(8, 128, 512).transpose(1, 0, 2)
        in_maps.append({
            "qt": ash(xl(np.asarray(Q)[b].T)),
            "kt": ash(xl(np.asarray(K)[b].T)),
            "vt": ash(np.asarray(V)[b].T.reshape(8, 128, 16, 128)
                      .transpose(2, 1, 0, 3)),
            "wqt": ash(wl(np.asarray(Wq)[jsl].T * np.float32(SCQ))),
            "wkt": ash(wl(np.asarray(Wk)[jsl].T)),
            "wvt": ash(wl(np.asarray(Wv)[jsl].T)),
            "wot": ash(np.asarray(Wo)[:, jsl].T.reshape(4, 128, 1024)
                       .transpose(1, 0, 2)),
            "bq": asf(np.asarray(bq)[jsl].reshape(4, 128).T * np.float32(SCQ)),
            "bk": asf(np.asarray(bk)[jsl].reshape(4, 128).T),
            "bvb": asf(np.broadcast_to(np.asarray(bv)[jsl], (128, JC))),
        })
    return in_maps


def kernel(Q, K, V, Wq, bq, Wk, bk, Wv, bv, Wo, bo, _trace=False, _trace_kwargs=None):
    nc = _get_nc()
    in_maps = make_in_maps(Q, K, V, Wq, bq, Wk, bk, Wv, bv, Wo, bo)
    res = run_bass_kernel_spmd(
        nc, in_maps, core_ids=list(range(N_CORES)),
        trace=_trace, **(_trace_kwargs or {}),
    )
    parts = [res.results[c]["out"].astype(np.float32) for c in range(N_CORES)]
    bo_np = np.asarray(bo, dtype=np.float32)
    O = np.stack([parts[2 * b] + parts[2 * b + 1] + bo_np for b in range(4)])
    kernel.last_results = res
    return O.astype(np.float32)

